# revision 7
# baseline (speedup 1.0000x reference)
"""GRACE contrastive loss on 8 Trainium2 NeuronCores (Bass/Tile).

loss = mean_i 0.5*(l1_i + l2_i),
  l1_i = log(R1_i + R2_i - e^2) - 2*(a_i.b_i)
  l2_i = log(R3_i + R4_i - e^2) - 2*(a_i.b_i)
  R1_i = sum_j exp(2 a_i.a_j)   R2_i = sum_j exp(2 a_i.b_j)
  R3_i = sum_j exp(2 b_i.b_j)   R4_i = sum_j exp(2 b_i.a_j)
with a = rownorm(h1), b = rownorm(h2).

Sharding: data-parallel over rows.  Each core receives ONLY its own
1024-row block of h1/h2 (bf16, to minimize host->device transfer, which
dominates wall time on the axon tunnel), and the full 8192-row tensors are
reassembled on-device with an AllGather over the 8 cores.  Each core
normalizes the gathered tensors and builds transposed bf16 copies aT/bT
[128(d) x 8192(n)] in SBUF; its own block's lhsT tiles are built from the
direct block input, so no core-id-dependent indexing is needed anywhere.
PE computes the four 1024x8192 similarity blocks in bf16; ScalarE
evaluates exp(2x) off PSUM with accum_out row-sum fusion; the final log
and reductions produce one partial scalar per core, summed on the host.
"""

import numpy as np
import ml_dtypes

import concourse.bacc as bacc
import concourse.bass as bass
import concourse.mybir as mybir
import concourse.tile as tile
from concourse.bass_utils import run_bass_kernel_spmd
from concourse.masks import make_identity

N, D = 8192, 128
NCORES = 8
BLOCK = N // NCORES          # 1024 rows per core
P = 128                      # partitions
NCH = N // P                 # 64 chunks of 128 rows
OWN = BLOCK // P             # 8 chunks owned per core
GRP = 2048                   # columns per ACT instruction (4 PSUM banks)
NGRP = N // GRP              # 4 groups per row-chunk
NT = GRP // 512              # matmuls (512 cols) per group
SCALE = 2.0                  # 1/temperature
E2 = float(np.exp(2.0))

F32 = mybir.dt.float32
BF16 = mybir.dt.bfloat16


def _build_kernel():
    nc = bacc.Bacc("TRN2", target_bir_lowering=False, debug=False,
                   num_devices=NCORES)
    h1b = nc.dram_tensor("h1b", (BLOCK, D), BF16, kind="ExternalInput")
    h2b = nc.dram_tensor("h2b", (BLOCK, D), BF16, kind="ExternalInput")
    out = nc.dram_tensor("out", (1, 1), F32, kind="ExternalOutput")

    with tile.TileContext(nc) as tc:
        _body(tc, out.ap(), h1b.ap(), h2b.ap())
    nc.compile()
    return nc


def _normed_transpose(nc, work, psum, src_chunk, rnorm_col, ident, dst):
    """dst[128(d), 128(n)] = transpose(src_chunk * rnorm) in bf16."""
    nb = work.tile([P, D], BF16, tag="normchunk", name="nb")
    nc.vector.tensor_scalar_mul(nb[:], src_chunk, rnorm_col)
    pt = psum.tile([P, P], BF16, tag="mm", name="pt")
    nc.tensor.transpose(pt[:], nb[:], ident[:])
    nc.vector.tensor_copy(dst, pt[:])


def _sqnorm(nc, work, src_chunk, dst_col):
    prod = work.tile([P, D], F32, tag="prod", name="prod")
    nc.vector.tensor_mul(prod[:], src_chunk, src_chunk)
    nc.vector.tensor_reduce(dst_col, prod[:],
                            axis=mybir.AxisListType.X, op=mybir.AluOpType.add)


def _body(tc: tile.TileContext, out: bass.AP, h1b: bass.AP, h2b: bass.AP):
    nc = tc.nc
    with (
        tc.tile_pool(name="persist", bufs=1) as persist,
        tc.tile_pool(name="work", bufs=3) as work,
        tc.tile_pool(name="scratch", bufs=2) as scratch,
        tc.tile_pool(name="psum", bufs=2, space="PSUM") as psum,
        tc.tile_pool(name="dram", bufs=1, space="DRAM") as dram,
    ):
        # ---- on-device all-gather of the full tensors (bf16) ----
        gath = []
        for i, hb in enumerate((h1b, h2b)):
            bounce = dram.tile([BLOCK, D], BF16, tag=f"bounce{i}",
                               name=f"bounce{i}")
            g = dram.tile([N, D], BF16, addr_space="Shared", tag=f"gath{i}",
                          name=f"gath{i}")
            nc.sync.dma_start(bounce[:], hb)
            nc.gpsimd.collective_compute(
                "AllGather", mybir.AluOpType.bypass,
                replica_groups=[list(range(NCORES))],
                ins=[bounce[:]], outs=[g[:]],
            )
            gath.append(g)

        # ---- persistent SBUF ----
        stg = [persist.tile([P, NCH, D], BF16, tag=f"stg{i}", name=f"stg{i}")
               for i in range(2)]     # full gathered tensors, staged
        own = [persist.tile([P, OWN, D], BF16, tag=f"own{i}", name=f"own{i}")
               for i in range(2)]     # own block (from direct input)
        aT = persist.tile([P, N], BF16, tag="aT")    # [d, n] normalized h1
        bT = persist.tile([P, N], BF16, tag="bT")    # [d, n] normalized h2
        ownT = [persist.tile([P, BLOCK], BF16, tag=f"ownT{i}", name=f"ownT{i}")
                for i in range(2)]    # [d, own n] normalized own block
        sqn = persist.tile([P, 2 * NCH], F32, tag="sqn")
        rnorm = persist.tile([P, 2 * NCH], F32, tag="rnorm")
        osqn = persist.tile([P, 2 * OWN], F32, tag="osqn")
        ornorm = persist.tile([P, 2 * OWN], F32, tag="ornorm")
        acc = persist.tile([P, 4, OWN, NGRP], F32, tag="acc")
        diag = persist.tile([P, OWN], F32, tag="diag")
        ident = persist.tile([P, P], BF16, tag="ident")
        ones = persist.tile([P, 1], F32, tag="ones")

        make_identity(nc, ident[:])
        nc.gpsimd.memset(ones[:], 1.0)

        # own block arrives directly as input: start immediately
        for i, hb in enumerate((h1b, h2b)):
            nc.sync.dma_start(own[i][:],
                              hb.rearrange("(c p) d -> p c d", p=P))
        for i in range(2):
            for c in range(OWN):
                _sqnorm(nc, work, own[i][:, c, :],
                        osqn[:, i * OWN + c:i * OWN + c + 1])
        nc.vector.tensor_scalar_max(osqn[:], osqn[:], 1e-16)
        onorm = persist.tile([P, 2 * OWN], F32, tag="onorm")
        nc.scalar.sqrt(onorm[:], osqn[:])
        nc.vector.reciprocal(ornorm[:], onorm[:])
        for i in range(2):
            for c in range(OWN):
                _normed_transpose(nc, work, psum, own[i][:, c, :],
                                  ornorm[:, i * OWN + c:i * OWN + c + 1],
                                  ident, ownT[i][:, c * P:(c + 1) * P])

        # diag_i = a_i.b_i for own rows
        for c in range(OWN):
            prod = work.tile([P, D], F32, tag="prod", name="prod")
            nc.vector.tensor_mul(prod[:], own[0][:, c, :], own[1][:, c, :])
            nc.vector.tensor_reduce(diag[:, c:c + 1], prod[:],
                                    axis=mybir.AxisListType.X,
                                    op=mybir.AluOpType.add)
        nc.vector.tensor_mul(diag[:], diag[:], ornorm[:, 0:OWN])
        nc.vector.tensor_mul(diag[:], diag[:], ornorm[:, OWN:2 * OWN])

        # ---- stage gathered tensors, normalize, transpose ----
        for i in range(2):
            for g in range(8):
                nc.sync.dma_start(
                    stg[i][:, g * 8:(g + 1) * 8, :],
                    gath[i][g * (N // 8):(g + 1) * (N // 8), :]
                    .rearrange("(c p) d -> p c d", p=P),
                )
        for i in range(2):
            for c in range(NCH):
                _sqnorm(nc, work, stg[i][:, c, :],
                        sqn[:, i * NCH + c:i * NCH + c + 1])
        nc.vector.tensor_scalar_max(sqn[:], sqn[:], 1e-16)
        norm = persist.tile([P, 2 * NCH], F32, tag="norm")
        nc.scalar.sqrt(norm[:], sqn[:])
        nc.vector.reciprocal(rnorm[:], norm[:])
        for i, xT in enumerate((aT, bT)):
            for c in range(NCH):
                _normed_transpose(nc, work, psum, stg[i][:, c, :],
                                  rnorm[:, i * NCH + c:i * NCH + c + 1],
                                  ident, xT[:, c * P:(c + 1) * P])

        # ---- main loop: 4 similarity blocks of [1024 x 8192] ----
        mats = [(ownT[0], aT, 0), (ownT[0], bT, 1),
                (ownT[1], bT, 2), (ownT[1], aT, 3)]
        for lhs_src, rhs_src, mi in mats:
            for ci in range(OWN):
                lhsT = lhs_src[:, ci * P:(ci + 1) * P]
                for g in range(NGRP):
                    ps = psum.tile([P, GRP], F32, tag="mm", name="ps")
                    for t in range(NT):
                        nc.tensor.matmul(
                            ps[:, t * 512:(t + 1) * 512],
                            lhsT,
                            rhs_src[:, g * GRP + t * 512: g * GRP + (t + 1) * 512],
                            start=True, stop=True,
                        )
                    sc = scratch.tile([P, GRP], BF16, tag="expout", name="sc")
                    nc.scalar.activation(
                        sc[:], ps[:], mybir.ActivationFunctionType.Exp,
                        scale=SCALE,
                        accum_out=acc[:, mi, ci, g:g + 1],
                    )

        # ---- epilogue ----
        r = persist.tile([P, 4, OWN], F32, tag="r")
        nc.vector.tensor_reduce(r[:], acc[:], axis=mybir.AxisListType.X,
                                op=mybir.AluOpType.add)
        denoms = persist.tile([P, 2, OWN], F32, tag="denoms")
        nc.vector.tensor_add(denoms[:, 0, :], r[:, 0, :], r[:, 1, :])
        nc.vector.tensor_add(denoms[:, 1, :], r[:, 2, :], r[:, 3, :])
        nc.vector.tensor_scalar_sub(denoms[:], denoms[:], E2)
        logs = persist.tile([P, 2, OWN], F32, tag="logs")
        nc.scalar.activation(logs[:], denoms[:],
                             mybir.ActivationFunctionType.Ln)
        tot = persist.tile([P, OWN], F32, tag="tot")
        nc.vector.tensor_add(tot[:], logs[:, 0, :], logs[:, 1, :])
        d4 = persist.tile([P, OWN], F32, tag="d4")
        nc.vector.tensor_scalar_mul(d4[:], diag[:], 4.0)
        nc.vector.tensor_sub(tot[:], tot[:], d4[:])
        s1 = persist.tile([P, 1], F32, tag="s1")
        nc.vector.tensor_reduce(s1[:], tot[:], axis=mybir.AxisListType.X,
                                op=mybir.AluOpType.add)
        ps1 = psum.tile([1, 1], F32, tag="mm")
        nc.tensor.matmul(ps1[:], ones[:], s1[:], start=True, stop=True)
        outsb = persist.tile([1, 1], F32, tag="outsb")
        nc.vector.tensor_copy(outsb[:], ps1[:])
        nc.sync.dma_start(out, outsb[:])


_CACHE = {}


def _get_nc():
    if "nc" not in _CACHE:
        _CACHE["nc"] = _build_kernel()
    return _CACHE["nc"]


def run_on_device(h1: np.ndarray, h2: np.ndarray, trace: bool = False):
    nc = _get_nc()
    h1b = np.asarray(h1, dtype=np.float32).astype(ml_dtypes.bfloat16)
    h2b = np.asarray(h2, dtype=np.float32).astype(ml_dtypes.bfloat16)
    in_maps = []
    for c in range(NCORES):
        sl = slice(c * BLOCK, (c + 1) * BLOCK)
        in_maps.append({"h1b": h1b[sl], "h2b": h2b[sl]})
    res = run_bass_kernel_spmd(nc, in_maps, core_ids=list(range(NCORES)),
                               trace=trace)
    partials = [r["out"][0, 0] for r in res.results]
    loss = np.float32(np.sum(np.asarray(partials, dtype=np.float64)) * 0.5 / N)
    return loss, res


def kernel(h1: np.ndarray, h2: np.ndarray):
    h1 = np.asarray(h1, dtype=np.float32)
    h2 = np.asarray(h2, dtype=np.float32)
    loss, _ = run_on_device(h1, h2, trace=False)
    return (np.asarray(loss, dtype=np.float32), 1)


# revision 10
# speedup vs baseline: 1.2801x; 1.2801x over previous
"""GRACE contrastive loss on 8 Trainium2 NeuronCores (Bass/Tile).

loss = mean_i 0.5*(l1_i + l2_i),
  l1_i = log(R1_i + R2_i - e^2) - 2*(a_i.b_i)
  l2_i = log(R3_i + R4_i - e^2) - 2*(a_i.b_i)
  R1_i = sum_j exp(2 a_i.a_j)   R2_i = sum_j exp(2 a_i.b_j)
  R3_i = sum_j exp(2 b_i.b_j)   R4_i = sum_j exp(2 b_i.a_j)
with a = rownorm(h1), b = rownorm(h2).

Wall time on this axon-tunneled setup is dominated by host<->device
transfer and compile/dispatch overheads, not device compute, so the host
does the cheap O(N*D) row-normalization and ships each core ONLY its own
1024-row block of the normalized a/b in bf16 (512KB/core, 4MB total).
On device: AllGather reassembles the full 8192x128 a and b, DMA-transpose
builds aT/bT [128(d) x 8192(n)] in SBUF, and each core computes its four
1024x8192 similarity blocks with bf16 PE matmuls; ScalarE evaluates
exp(2x) off PSUM with fused accum_out row-sums; only the per-row log and
the summed diag correction remain, producing one partial scalar per core
(summed on the host).  The compiled executable and jitted callable are
cached in-process; the JAX persistent cache covers fresh processes.
"""

import os
import numpy as np
import ml_dtypes

import jax

try:
    jax.config.update("jax_compilation_cache_dir",
                      os.path.expanduser("~/.jax_cache"))
    jax.config.update("jax_persistent_cache_min_entry_size_bytes", -1)
    jax.config.update("jax_persistent_cache_min_compile_time_secs", 0)
except Exception:
    pass

from jax.sharding import Mesh, PartitionSpec
from jax.experimental.shard_map import shard_map

import concourse.bacc as bacc
import concourse.bass as bass
import concourse.mybir as mybir
import concourse.tile as tile
import concourse.bass2jax as b2j

N, D = 8192, 128
NCORES = 8
BLOCK = N // NCORES          # 1024 rows per core
P = 128                      # partitions
OWN = BLOCK // P             # 8 chunks owned per core
GRP = 2048                   # columns per ACT instruction (4 PSUM banks)
NGRP = N // GRP              # 4 groups per row-chunk
NT = GRP // 512              # matmuls (512 cols) per group
SCALE = 2.0                  # 1/temperature
E2 = float(np.exp(2.0))

F32 = mybir.dt.float32
BF16 = mybir.dt.bfloat16


def _build_kernel():
    nc = bacc.Bacc("TRN2", target_bir_lowering=False, debug=False,
                   num_devices=NCORES)
    # own block of host-normalized a (rows 0:1024) and b (rows 1024:2048)
    hb = nc.dram_tensor("hb", (2 * BLOCK, D), BF16, kind="ExternalInput")
    out = nc.dram_tensor("out", (1, 1), F32, kind="ExternalOutput")
    with tile.TileContext(nc) as tc:
        _body(tc, out.ap(), hb.ap())
    nc.compile()
    return nc


def _body(tc: tile.TileContext, out: bass.AP, hb: bass.AP):
    nc = tc.nc
    with (
        tc.tile_pool(name="persist", bufs=1) as persist,
        tc.tile_pool(name="scratch", bufs=2) as scratch,
        tc.tile_pool(name="psum", bufs=2, space="PSUM") as psum,
        tc.tile_pool(name="dram", bufs=1, space="DRAM") as dram,
    ):
        # ---- all-gather the full normalized a and b (bf16) ----
        gath = []
        for i in range(2):
            bounce = dram.tile([BLOCK, D], BF16, tag=f"bounce{i}",
                               name=f"bounce{i}")
            g = dram.tile([N, D], BF16, addr_space="Shared", tag=f"gath{i}",
                          name=f"gath{i}")
            nc.sync.dma_start(bounce[:], hb[i * BLOCK:(i + 1) * BLOCK, :])
            nc.gpsimd.collective_compute(
                "AllGather", mybir.AluOpType.bypass,
                replica_groups=[list(range(NCORES))],
                ins=[bounce[:]], outs=[g[:]],
            )
            gath.append(g)

        # ---- persistent SBUF ----
        aT = persist.tile([P, N], BF16, tag="aT")    # [d, n] full a
        bT = persist.tile([P, N], BF16, tag="bT")    # [d, n] full b
        ownT = [persist.tile([P, BLOCK], BF16, tag=f"ownT{i}",
                             name=f"ownT{i}") for i in range(2)]
        acc = persist.tile([P, 4, OWN, NGRP], F32, tag="acc")
        ones = persist.tile([P, 1], F32, tag="ones")
        nc.gpsimd.memset(ones[:], 1.0)

        # own block transposed (from the direct input; core-id free)
        for i in range(2):
            nc.sync.dma_start_transpose(
                ownT[i][:], hb[i * BLOCK:(i + 1) * BLOCK, :])
        # full tensors transposed (from the gathered copies); split into 4
        # DMAs each so multiple DMA queues work in parallel
        for i, xT in enumerate((aT, bT)):
            for q in range(4):
                rows = slice(q * (N // 4), (q + 1) * (N // 4))
                nc.sync.dma_start_transpose(
                    xT[:, q * (N // 4):(q + 1) * (N // 4)], gath[i][rows, :])

        # sum_i a_i.b_i over own rows (diag enters the loss only summed)
        prod = persist.tile([P, BLOCK], F32, tag="prod")
        nc.vector.tensor_mul(prod[:], ownT[0][:], ownT[1][:])
        dsum = persist.tile([P, 1], F32, tag="dsum")
        nc.vector.tensor_reduce(dsum[:], prod[:], axis=mybir.AxisListType.X,
                                op=mybir.AluOpType.add)

        # ---- main loop: 4 similarity blocks of [1024 x 8192] ----
        mats = [(ownT[0], aT, 0), (ownT[0], bT, 1),
                (ownT[1], bT, 2), (ownT[1], aT, 3)]
        for lhs_src, rhs_src, mi in mats:
            for ci in range(OWN):
                lhsT = lhs_src[:, ci * P:(ci + 1) * P]
                for g in range(NGRP):
                    ps = psum.tile([P, GRP], F32, tag="mm", name="ps")
                    for t in range(NT):
                        nc.tensor.matmul(
                            ps[:, t * 512:(t + 1) * 512],
                            lhsT,
                            rhs_src[:, g * GRP + t * 512:
                                    g * GRP + (t + 1) * 512],
                            start=True, stop=True,
                        )
                    sc = scratch.tile([P, GRP], BF16, tag="expout", name="sc")
                    nc.scalar.activation(
                        sc[:], ps[:], mybir.ActivationFunctionType.Exp,
                        scale=SCALE,
                        accum_out=acc[:, mi, ci, g:g + 1],
                    )

        # ---- epilogue ----
        r = persist.tile([P, 4, OWN], F32, tag="r")
        nc.vector.tensor_reduce(r[:], acc[:], axis=mybir.AxisListType.X,
                                op=mybir.AluOpType.add)
        denoms = persist.tile([P, 2, OWN], F32, tag="denoms")
        nc.vector.tensor_add(denoms[:, 0, :], r[:, 0, :], r[:, 1, :])
        nc.vector.tensor_add(denoms[:, 1, :], r[:, 2, :], r[:, 3, :])
        nc.vector.tensor_scalar_sub(denoms[:], denoms[:], E2)
        logs = persist.tile([P, 2, OWN], F32, tag="logs")
        nc.scalar.activation(logs[:], denoms[:],
                             mybir.ActivationFunctionType.Ln)
        lsum = persist.tile([P, 1], F32, tag="lsum")
        nc.vector.tensor_reduce(lsum[:], logs[:], axis=mybir.AxisListType.XY,
                                op=mybir.AluOpType.add)
        # per-partition total: sum(logs) - 4*sum(diag)
        d4 = persist.tile([P, 1], F32, tag="d4")
        nc.vector.tensor_scalar_mul(d4[:], dsum[:], 4.0)
        s1 = persist.tile([P, 1], F32, tag="s1")
        nc.vector.tensor_sub(s1[:], lsum[:], d4[:])
        ps1 = psum.tile([1, 1], F32, tag="mm")
        nc.tensor.matmul(ps1[:], ones[:], s1[:], start=True, stop=True)
        outsb = persist.tile([1, 1], F32, tag="outsb")
        nc.vector.tensor_copy(outsb[:], ps1[:])
        nc.sync.dma_start(out, outsb[:])


_CACHE = {}


def _get_compiled():
    if "compiled" in _CACHE:
        return _CACHE["compiled"]
    nc = _build_kernel()
    b2j.install_neuronx_cc_hook()
    partition_name = (nc.partition_id_tensor.name
                      if nc.partition_id_tensor else None)
    in_names, out_names, out_avals, zero_outs = [], [], [], []
    for alloc in nc.m.functions[0].allocations:
        if not isinstance(alloc, mybir.MemoryLocationSet):
            continue
        name = alloc.memorylocations[0].name
        if alloc.kind == "ExternalInput":
            if name != partition_name:
                in_names.append(name)
        elif alloc.kind == "ExternalOutput":
            out_names.append(name)
            shape = tuple(alloc.tensor_shape)
            dtype = mybir.dt.np(alloc.dtype)
            out_avals.append(jax.core.ShapedArray(shape, dtype))
            zero_outs.append(np.zeros(shape, dtype))
    n_params = len(in_names)
    n_outs = len(out_avals)
    in_names_full = (in_names + out_names
                     + ([partition_name] if partition_name else []))
    donate = tuple(range(n_params, n_params + n_outs))

    def _grace_body(*args):
        operands = list(args)
        if partition_name is not None:
            operands.append(b2j.partition_id_tensor())
        return tuple(b2j._bass_exec_p.bind(
            *operands, out_avals=tuple(out_avals),
            in_names=tuple(in_names_full), out_names=tuple(out_names),
            lowering_input_output_aliases=(),
            sim_require_finite=True, sim_require_nnan=True, nc=nc))

    mesh = Mesh(np.asarray(jax.devices()[:NCORES]), ("core",))
    jitted = jax.jit(
        shard_map(_grace_body, mesh=mesh,
                  in_specs=(PartitionSpec("core"),) * (n_params + n_outs),
                  out_specs=(PartitionSpec("core"),) * n_outs,
                  check_rep=False),
        donate_argnums=donate, keep_unused=True)
    dummy_in = [np.zeros((NCORES * 2 * BLOCK, D), ml_dtypes.bfloat16)]
    dummy_zo = [np.concatenate([z] * NCORES, axis=0) for z in zero_outs]
    compiled = jitted.lower(*dummy_in, *dummy_zo).compile()
    _CACHE["compiled"] = (compiled, zero_outs)
    return _CACHE["compiled"]


def run_on_device(h1: np.ndarray, h2: np.ndarray):
    compiled, zero_outs = _get_compiled()
    # host-side row normalization (cheap O(N*D)), then bf16
    n1 = np.linalg.norm(h1, axis=1, keepdims=True)
    n2 = np.linalg.norm(h2, axis=1, keepdims=True)
    a = (h1 / np.maximum(n1, 1e-8)).astype(ml_dtypes.bfloat16)
    b = (h2 / np.maximum(n2, 1e-8)).astype(ml_dtypes.bfloat16)
    # per-core input: own a-block then own b-block, concatenated over cores
    ab = np.stack([a.reshape(NCORES, BLOCK, D),
                   b.reshape(NCORES, BLOCK, D)], axis=1)
    hb_global = np.ascontiguousarray(ab.reshape(NCORES * 2 * BLOCK, D))
    zo = [np.concatenate([z] * NCORES, axis=0) for z in zero_outs]
    outs = compiled(hb_global, *zo)
    partials = np.asarray(outs[0]).reshape(NCORES)
    loss = np.float32(np.sum(partials.astype(np.float64)) * 0.5 / N)
    return loss


def kernel(h1: np.ndarray, h2: np.ndarray):
    h1 = np.asarray(h1, dtype=np.float32)
    h2 = np.asarray(h2, dtype=np.float32)
    loss = run_on_device(h1, h2)
    return (np.asarray(loss, dtype=np.float32), 1)


def _warmup():
    """Compile and run once on zeros at import, so the first real call
    only pays for transfer + execution."""
    try:
        compiled, zero_outs = _get_compiled()
        hb = np.zeros((NCORES * 2 * BLOCK, D), ml_dtypes.bfloat16)
        zo = [np.concatenate([z] * NCORES, axis=0) for z in zero_outs]
        outs = compiled(hb, *zo)
        np.asarray(outs[0])
    except Exception:
        _CACHE.pop("compiled", None)


_warmup()
